# revision 13
# baseline (speedup 1.0000x reference)
"""Trainium2 Bass kernel for DenseGraphSimpleOpEdgeFlow (GNN message passing).

Reference semantics (per batch b):
  support = inputs @ weight                                    [N, F]
  op_emb[diag] = self_op_emb; adjP = adj + I
  attn = sigmoid(op_emb @ attn_w + attn_b)                     [N, N, F]
  attn = (adjP != 0) * attn;  attn = 1 where adjP == 1 (skip)
  out[i, :] = sum_j attn[i, j, :] * support[j, :] + support[i, :]

Sharding: data-parallel over batch B=64 across 8 cores (8 batches/core).

Per-core device mapping (engine-balanced around the ACT sigmoid wall):
  * z = W_aug^T @ op_emb^T on TensorE, bf16 moving tensor marshaled on the
    host (49 rows = 48 op dims + one mask-logit row in {0,-100} which drives
    sigmoid to 0 for masked and skip edges). Two 49-row halves at partition
    bases 0/64 cover each batch's 9216 edge columns.
  * sigmoid on ScalarE straight out of PSUM (bias = attn_b per partition),
    bf16 out. 48 x 1536-col instructions: this engine is the wall.
  * DVE (2x bf16): prod = sigma * support_bcast per half (4608 cols), then
    tree levels L1 (96j->48) and L2 (48->24).
  * Pool (GpSimd): tree tail L3/L4/L5 (24->12->6->3) and the three final
    48-col adds (3-way cred + s1), writing bf16 out_fin.
  * skip/identity term S1[d,(b,i)] = support^T @ (skipmask+I) as small
    TensorE matmuls; skim mask is host-marshaled (bf16), so no on-device
    mask arithmetic at all.
  * output stays [d, (b,i)] bf16 and is DMA'd per batch; the host undoes
    the transpose. No on-device output transposes.
  * all DMA triggers issue from the Sync (SP) queue, keeping Pool free.
"""

import numpy as np

B, N, IN_F, OUT_F, OP_D = 64, 96, 128, 128, 48
NCORES = 8
BPC = B // NCORES  # batches per core
HALF = (N // 2) * N  # 4608 columns per half
NEG = -100.0

_CACHE = {}

# columns of each batch's multiply handled by Pool (0 disables)
POOL_MULT_COLS = 0
# i-groups (of 48 per half) of the L1 tree level handled by Pool
POOL_L1_GROUPS = 16


def _build_nc():
    import concourse.bass as bass
    import concourse.bacc as bacc
    import concourse.tile as tile
    from concourse import mybir
    from contextlib import ExitStack

    f32 = mybir.dt.float32
    bf16 = mybir.dt.bfloat16
    MUL = mybir.AluOpType.mult
    ADD = mybir.AluOpType.add

    nc = bacc.Bacc(None, target_bir_lowering=False)

    # ---- DRAM parameters (per-core shard views, host-marshaled) ----
    # op4[b, h] is [49, HALF] bf16: rows 0-47 = op_emb^T (diagonal replaced
    # host-side), row 48 = mask logit row in {0, -100}.
    p_op4 = nc.declare_dram_parameter("op4", [BPC, 2, OP_D + 1, HALF], bf16,
                                      isOutput=False)
    # skim[j, b, i] = (adjP == 1) + I : skip edges plus residual identity
    p_skim = nc.declare_dram_parameter("skim", [N, BPC, N], bf16, isOutput=False)
    # packed constants: pbf = [w2 | wgt | inpt]
    p_pbf = nc.declare_dram_parameter("pbf", [128, 1024], bf16, isOutput=False)
    p_attnb = nc.declare_dram_parameter("attnb", [128, 1], f32, isOutput=False)
    p_out = nc.declare_dram_parameter("out", [OUT_F, BPC * N], bf16, isOutput=True)

    NB = BPC * N  # 768

    def sub_ap(ap, extra_off, dims):
        return bass.AP(tensor=ap.tensor, offset=ap.offset + extra_off,
                       ap=[ap.ap[0]] + dims)

    with tile.TileContext(nc) as tc, ExitStack() as ctx:
        const = ctx.enter_context(tc.tile_pool(name="const", bufs=1))
        rhs_pool = ctx.enter_context(tc.tile_pool(name="rhs", bufs=4))

        SIG = mybir.ActivationFunctionType.Sigmoid

        # const loads (SP queue)
        pbf_sb = const.tile([128, 1024], bf16)
        nc.sync.dma_start(out=pbf_sb[:], in_=p_pbf[:, :])
        attnb_sb = const.tile([128, 1], f32)
        nc.sync.dma_start(out=attnb_sb[:], in_=p_attnb[:, :])
        skim_sb = const.tile([N, BPC, N], bf16)
        nc.sync.dma_start(out=skim_sb[:], in_=p_skim[:, :, :])

        w2_sb = pbf_sb[:, 0:128]
        wgt_sb = pbf_sb[:, 128:256]
        inpt_sb = pbf_sb[:, 256:1024]

        # sigmoid ACT-table warm: first ACT instruction is a sigmoid, so the
        # one table load happens during the DMA ramp, not mid-pipeline
        warm_sb = const.tile([OUT_F, 1], bf16)
        nc.scalar.activation(out=warm_sb[:], in_=attnb_sb[:], func=SIG)

        # big moving-tensor loads
        rts = [None] * BPC

        def load_rt(b, split=False):
            rt = rhs_pool.tile([128, HALF], bf16, tag="rt")
            if split:
                # chunk-granular load so the first matmuls start earlier
                for h in range(2):
                    for k in range(3):
                        c0 = k * 1536
                        nc.gpsimd.dma_start(
                            out=rt[64 * h:64 * h + OP_D + 1, c0:c0 + 1536],
                            in_=p_op4[b, h][:, c0:c0 + 1536])
            else:
                nc.gpsimd.dma_start(out=rt[0:OP_D + 1, :], in_=p_op4[b, 0])
                nc.gpsimd.dma_start(out=rt[64:64 + OP_D + 1, :], in_=p_op4[b, 1])
            rts[b] = rt

        load_rt(0, split=True)

        stbf_sb = const.tile([OUT_F, NB], bf16)       # support^T in bf16
        s1_sb = const.tile([OUT_F, NB], bf16)         # skip+identity term
        snat_sb = const.tile([N, BPC, OUT_F], bf16)   # support natural [j, b, d]

        pz = ctx.enter_context(tc.tile_pool(name="pz", bufs=2, space="PSUM"))
        ptr = ctx.enter_context(tc.tile_pool(name="ptr", bufs=2, space="PSUM"))

        def zslot():
            zt = pz.tile([OUT_F, 1536], f32, tag="z")
            return zt

        # ---------------- pre-phase: support, S1 ----------------
        # support^T [d, (b,j)] = weight^T @ inputs^T
        stp = zslot()
        nc.tensor.matmul(stp[:, 0:512], lhsT=wgt_sb,
                         rhs=inpt_sb[:, 0:512], start=True, stop=True)
        nc.tensor.matmul(stp[:, 512:NB], lhsT=wgt_sb,
                         rhs=inpt_sb[:, 512:NB], start=True, stop=True)
        nc.scalar.copy(out=stbf_sb[:], in_=stp[:, 0:NB])

        def emit_snat(b):
            # support natural [j, d] per b (stationary for the S1 matmul)
            pn = ptr.tile([128, 128], f32, tag="pt")
            nc.tensor.matmul(pn[:N, 0:OUT_F], lhsT=inpt_sb[:, b * N:(b + 1) * N],
                             rhs=wgt_sb, start=True, stop=True)
            nc.scalar.copy(out=snat_sb[:, b, :], in_=pn[:N, 0:OUT_F])

        def emit_s1(b):
            # S1[d, (b,i)] = sum_j support[j, d] * (skip+I)[j, i]
            ps1 = ptr.tile([128, 128], f32, tag="pt")
            nc.tensor.matmul(ps1[:, 0:N], lhsT=snat_sb[:, b, :],
                             rhs=skim_sb[:, b, :], start=True, stop=True)
            nc.scalar.copy(out=s1_sb[:, b * N:(b + 1) * N], in_=ps1[:, 0:N])

        load_rt(1)
        load_rt(2)
        load_rt(3)

        # ---------------- main loop over batches ----------------
        sig_pool = ctx.enter_context(tc.tile_pool(name="sig", bufs=3))
        prod_pool = ctx.enter_context(tc.tile_pool(name="prod", bufs=3))
        l1_pool = ctx.enter_context(tc.tile_pool(name="l1", bufs=2))
        l2_pool = ctx.enter_context(tc.tile_pool(name="l2", bufs=2))
        l3_pool = ctx.enter_context(tc.tile_pool(name="l3", bufs=2))
        l4_pool = ctx.enter_context(tc.tile_pool(name="l4", bufs=2))
        l5_pool = ctx.enter_context(tc.tile_pool(name="l5", bufs=2))
        crd_pool = ctx.enter_context(tc.tile_pool(name="crd", bufs=2))
        ofin = ctx.enter_context(tc.tile_pool(name="ofin", bufs=1))

        out_fin = ofin.tile([OUT_F, NB], bf16)

        NH = N // 2  # 48 i-rows per half

        for b in range(BPC):
            if b + 4 < BPC:
                load_rt(b + 4)
            rt = rts[b]

            st_b = stbf_sb[:, b * N:(b + 1) * N]
            l1 = l1_pool.tile([OUT_F, N * 48], bf16)

            for h in range(2):
                pbase = 64 * h
                sig_t = sig_pool.tile([OUT_F, HALF], bf16)
                for k in range(3):
                    pzt = zslot()
                    for s in range(3):
                        cc = k * 1536 + s * 512
                        nc.tensor.matmul(
                            pzt[:, s * 512:(s + 1) * 512],
                            lhsT=w2_sb[pbase:pbase + OP_D + 1, :],
                            rhs=rt[pbase:pbase + OP_D + 1, cc:cc + 512],
                            start=True, stop=True)
                    nc.scalar.activation(out=sig_t[:, k * 1536:(k + 1) * 1536],
                                         in_=pzt[:], func=SIG,
                                         bias=attnb_sb[:], scale=1.0)

                # prod[d, (i, j)] = sigma * support_bcast; Pool takes the
                # trailing POOL_MULT_COLS of the h=1 half to relieve DVE
                prod = prod_pool.tile([OUT_F, HALF], bf16)
                pmc = POOL_MULT_COLS if h == 1 else 0
                dvc = HALF - pmc
                ndv = dvc // N
                st_bcast = bass.AP(tensor=st_b.tensor, offset=st_b.offset,
                                   ap=[st_b.ap[0], [0, ndv], st_b.ap[1]])
                nc.vector.tensor_tensor(out=prod[:, 0:dvc],
                                        in0=sig_t[:, 0:dvc],
                                        in1=st_bcast, op=MUL)
                if pmc:
                    st_bcast_p = bass.AP(tensor=st_b.tensor, offset=st_b.offset,
                                         ap=[st_b.ap[0], [0, pmc // N],
                                             st_b.ap[1]])
                    nc.gpsimd.tensor_tensor(out=prod[:, dvc:HALF],
                                            in0=sig_t[:, dvc:HALF],
                                            in1=st_bcast_p, op=MUL)

                # L1: j 96 -> 48 (per half, into the shared batch tile);
                # the trailing POOL_L1_GROUPS i-groups go to Pool
                ng = NH - POOL_L1_GROUPS
                nc.vector.tensor_tensor(
                    out=l1[:, h * NH * 48:h * NH * 48 + ng * 48],
                    in0=sub_ap(prod[:], 0, [[N, ng], [1, 48]]),
                    in1=sub_ap(prod[:], 48, [[N, ng], [1, 48]]),
                    op=ADD)
                if POOL_L1_GROUPS:
                    nc.gpsimd.tensor_tensor(
                        out=l1[:, h * NH * 48 + ng * 48:(h + 1) * NH * 48],
                        in0=sub_ap(prod[:], ng * N, [[N, POOL_L1_GROUPS], [1, 48]]),
                        in1=sub_ap(prod[:], ng * N + 48,
                                   [[N, POOL_L1_GROUPS], [1, 48]]),
                        op=ADD)

                if b == 0 and h == 0:
                    # pre-phase tail: S1 matmuls + copies fill early ACT
                    # bubbles without gating the first sigmoids
                    for bb in range(BPC):
                        emit_snat(bb)
                    for bb in range(BPC):
                        emit_s1(bb)

            # remaining levels once per batch over all 96 i-groups
            l2 = l2_pool.tile([OUT_F, N * 24], bf16)
            nc.vector.tensor_tensor(
                out=l2[:],
                in0=sub_ap(l1[:], 0, [[48, N], [1, 24]]),
                in1=sub_ap(l1[:], 24, [[48, N], [1, 24]]),
                op=ADD)
            l3 = l3_pool.tile([OUT_F, N * 12], bf16)
            nc.vector.tensor_tensor(
                out=l3[:],
                in0=sub_ap(l2[:], 0, [[24, N], [1, 12]]),
                in1=sub_ap(l2[:], 12, [[24, N], [1, 12]]),
                op=ADD)
            l4 = l4_pool.tile([OUT_F, N * 6], bf16)
            nc.vector.tensor_tensor(
                out=l4[:],
                in0=sub_ap(l3[:], 0, [[12, N], [1, 6]]),
                in1=sub_ap(l3[:], 6, [[12, N], [1, 6]]),
                op=ADD)
            l5 = l5_pool.tile([OUT_F, N * 3], bf16)
            nc.vector.tensor_tensor(
                out=l5[:],
                in0=sub_ap(l4[:], 0, [[6, N], [1, 3]]),
                in1=sub_ap(l4[:], 3, [[6, N], [1, 3]]),
                op=ADD)
            # finals: cred = l5a + l5b + l5c ; out = cred + s1
            o0 = b * N
            crd = crd_pool.tile([OUT_F, N], bf16)
            nc.vector.tensor_tensor(
                out=crd[:],
                in0=sub_ap(l5[:], 0, [[3, N]]),
                in1=sub_ap(l5[:], 1, [[3, N]]),
                op=ADD)
            crd2 = crd_pool.tile([OUT_F, N], bf16, tag="crd2")
            nc.vector.tensor_tensor(
                out=crd2[:],
                in0=crd[:],
                in1=sub_ap(l5[:], 2, [[3, N]]),
                op=ADD)
            nc.vector.tensor_tensor(
                out=out_fin[:, o0:o0 + N],
                in0=crd2[:],
                in1=s1_sb[:, o0:o0 + N],
                op=ADD)

            # store this batch's output columns
            nc.sync.dma_start(out=p_out[:, b * N:(b + 1) * N],
                              in_=out_fin[:, b * N:(b + 1) * N])

    nc.finalize()
    return nc


def _get_nc():
    if "nc" not in _CACHE:
        _CACHE["nc"] = _build_nc()
    return _CACHE["nc"]


def marshal_core(inputs, adj, op_emb, weight, attn_w, attn_b, self_op_emb, core):
    """Build the in_map for one core (layout/dtype marshaling + mask logits)."""
    from ml_dtypes import bfloat16

    sl = slice(core * BPC, (core + 1) * BPC)
    op_sh = np.array(op_emb[sl], np.float32)              # [BPC, N, N, OP_D]
    idx = np.arange(N)
    op_sh[:, idx, idx, :] = np.asarray(self_op_emb, np.float32)
    op_t = op_sh.transpose(0, 3, 1, 2)                    # [BPC, OP_D, N(i), N(j)]
    adj_sh = np.asarray(adj[sl]).astype(np.int32)         # [BPC, N, N]
    eye = np.eye(N, dtype=np.float32)
    # mask logit row: -100 where (adj + I) in {0, 1} else 0
    adjp = adj_sh.astype(np.float32) + eye
    m2 = np.where(adjp <= 1.0, np.float32(NEG), np.float32(0.0))  # [BPC, N, N]
    op4 = np.empty((BPC, 2, OP_D + 1, HALF), bfloat16)
    op4[:, :, :OP_D, :] = op_t.reshape(BPC, OP_D, 2, HALF).transpose(
        0, 2, 1, 3).astype(bfloat16)
    op4[:, :, OP_D, :] = m2.reshape(BPC, 2, HALF).astype(bfloat16)
    # skim[j, b, i] = (adjP == 1) + I, i.e. skip edges + residual identity
    skim = ((adjp == 1.0).astype(np.float32) + eye)        # [BPC, N(i), N(j)]
    skim = np.ascontiguousarray(skim.transpose(2, 0, 1)).astype(bfloat16)
    inpt = np.ascontiguousarray(
        np.asarray(inputs[sl], np.float32).reshape(BPC * N, IN_F).T)

    w2 = np.zeros((128, 128), np.float32)
    w2[0:OP_D] = attn_w
    w2[OP_D] = 1.0
    w2[64:64 + OP_D] = attn_w
    w2[64 + OP_D] = 1.0

    pbf = np.zeros((128, 1024), bfloat16)
    pbf[:, 0:128] = w2.astype(bfloat16)
    pbf[:, 128:256] = np.asarray(weight, np.float32).astype(bfloat16)
    pbf[:, 256:1024] = inpt.astype(bfloat16)
    attnb = np.asarray(attn_b, np.float32).reshape(128, 1)

    return {
        "op4": op4,
        "skim": skim,
        "pbf": pbf,
        "attnb": attnb,
    }


def _ensure_ntff_hook():
    """Provide antenv.axon_hooks if the image lacks it (NTFF timing under axon)."""
    import sys as _sys

    try:
        from antenv.axon_hooks import get_axon_ntff_profile_hook  # noqa: F401
        return
    except ImportError:
        pass

    import contextlib
    import ctypes
    import types

    so_path = "/opt/axon/libaxon_pjrt.so"
    try:
        lib = ctypes.CDLL(so_path)
    except OSError:
        lib = None
    if lib is None or not hasattr(lib, "axon_start_nrt_profile"):
        hook = None
    else:
        lib.axon_start_nrt_profile.argtypes = [
            ctypes.POINTER(ctypes.c_int64), ctypes.c_size_t]
        lib.axon_start_nrt_profile.restype = ctypes.c_int64
        lib.axon_stop_nrt_profile.argtypes = [ctypes.c_char_p]
        lib.axon_stop_nrt_profile.restype = ctypes.c_int64

        @contextlib.contextmanager
        def hook(output_dir, device_ids):
            import jax
            jax.devices()
            if device_ids:
                ids = (ctypes.c_int64 * len(device_ids))(*device_ids)
                rc = lib.axon_start_nrt_profile(ids, len(device_ids))
            else:
                rc = lib.axon_start_nrt_profile(None, 0)
            if rc != 0:
                raise RuntimeError(f"axon_start_nrt_profile rc={rc}")
            try:
                yield
            finally:
                n = lib.axon_stop_nrt_profile(str(output_dir).encode())
                print(f"ntff profile: {n} file(s) written to {output_dir}")

    mod = types.ModuleType("antenv.axon_hooks")
    _state = {"hook": hook}
    mod.get_axon_ntff_profile_hook = lambda: _state["hook"]

    def _set(h):
        _state["hook"] = h

    mod.set_axon_ntff_profile_hook = _set
    _sys.modules["antenv.axon_hooks"] = mod


def run(inputs, adj, op_emb, weight, attn_w, attn_b, self_op_emb, trace=False):
    if trace:
        _ensure_ntff_hook()
    from concourse.bass_utils import run_bass_kernel_spmd

    nc = _get_nc()
    in_maps = [
        marshal_core(inputs, adj, op_emb, weight, attn_w, attn_b, self_op_emb, c)
        for c in range(NCORES)
    ]
    res = run_bass_kernel_spmd(nc, in_maps, core_ids=list(range(NCORES)), trace=trace)
    # out is [F, BPC*N] bf16 per core; undo the transpose on host
    out = np.concatenate(
        [np.asarray(res.results[c]["out"], np.float32)
         .reshape(OUT_F, BPC, N).transpose(1, 2, 0)
         for c in range(NCORES)], axis=0)
    return np.ascontiguousarray(out, np.float32), res


def kernel(inputs, adj, op_emb, weight, attn_w, attn_b, self_op_emb):
    out, _ = run(inputs, adj, op_emb, weight, attn_w, attn_b, self_op_emb, trace=False)
    return out


# revision 24
# speedup vs baseline: 1.1043x; 1.1043x over previous
"""Trainium2 Bass kernel for DenseGraphSimpleOpEdgeFlow (GNN message passing).

Reference semantics (per batch b):
  support = inputs @ weight                                    [N, F]
  op_emb[diag] = self_op_emb; adjP = adj + I
  attn = sigmoid(op_emb @ attn_w + attn_b)                     [N, N, F]
  attn = (adjP != 0) * attn;  attn = 1 where adjP == 1 (skip)
  out[i, :] = sum_j attn[i, j, :] * support[j, :] + support[i, :]

Sharding: data-parallel over batch B=64 across 8 cores (8 batches/core).

Per-core device mapping (engine-balanced around the ACT sigmoid wall):
  * z = W_aug^T @ op_emb^T on TensorE, bf16 moving tensor marshaled on the
    host (49 rows = 48 op dims + one mask-logit row in {0,-100} which drives
    sigmoid to 0 for masked and skip edges). Two 49-row halves at partition
    bases 0/64 cover each batch's 9216 edge columns.
  * sigmoid on ScalarE straight out of PSUM (bias = attn_b per partition),
    bf16 out. 48 x 1536-col instructions: this engine is the wall.
  * DVE (2x bf16): prod = sigma * support_bcast per half (4608 cols), then
    tree levels L1 (96j->48) and L2 (48->24).
  * Pool (GpSimd): tree tail L3/L4/L5 (24->12->6->3) and the three final
    48-col adds (3-way cred + s1), writing bf16 out_fin.
  * skip/identity term S1[d,(b,i)] = support^T @ (skipmask+I) as small
    TensorE matmuls; skim mask is host-marshaled (bf16), so no on-device
    mask arithmetic at all.
  * output stays [d, (b,i)] bf16 and is DMA'd per batch; the host undoes
    the transpose. No on-device output transposes.
  * all DMA triggers issue from the Sync (SP) queue, keeping Pool free.
"""

import numpy as np

B, N, IN_F, OUT_F, OP_D = 64, 96, 128, 128, 48
NCORES = 8
BPC = B // NCORES  # batches per core
HALF = (N // 2) * N  # 4608 columns per half
NEG = -100.0

_CACHE = {}

# columns of each batch's multiply handled by Pool (0 disables)
POOL_MULT_COLS = 0
# i-groups (of 48 per half) of the L1 tree level handled by Pool.
# NOTE: any Pool compute poisons DVE 2x throughput (shared SBUF port pair),
# so this must stay 0 while DVE is the bottleneck.
POOL_L1_GROUPS = 0
# tree level whose remaining j-width is accumulated on the PE via
# identity-matmuls into PSUM ('l2' = 24 views, 'l3' = 12, 'l4' = 6)
TAIL_FROM = "l3"


def _build_nc():
    import concourse.bass as bass
    import concourse.bacc as bacc
    import concourse.tile as tile
    from concourse import mybir
    from contextlib import ExitStack

    f32 = mybir.dt.float32
    bf16 = mybir.dt.bfloat16
    MUL = mybir.AluOpType.mult
    ADD = mybir.AluOpType.add

    nc = bacc.Bacc(None, target_bir_lowering=False)

    # ---- DRAM parameters (per-core shard views, host-marshaled) ----
    # op4[b, h] is [49, HALF] bf16: rows 0-47 = op_emb^T (diagonal replaced
    # host-side), row 48 = mask logit row in {0, -100}.
    p_op4 = nc.declare_dram_parameter("op4", [BPC, 2, OP_D + 1, HALF], bf16,
                                      isOutput=False)
    # skim[j, b, i] = (adjP == 1) + I : skip edges plus residual identity
    p_skim = nc.declare_dram_parameter("skim", [N, BPC, N], bf16, isOutput=False)
    # packed constants: pbf = [w2 | wgt | inpt | ident]
    p_pbf = nc.declare_dram_parameter("pbf", [128, 1152], bf16, isOutput=False)
    p_attnb = nc.declare_dram_parameter("attnb", [128, 1], f32, isOutput=False)
    p_out = nc.declare_dram_parameter("out", [OUT_F, BPC * N], bf16, isOutput=True)

    NB = BPC * N  # 768

    def sub_ap(ap, extra_off, dims):
        return bass.AP(tensor=ap.tensor, offset=ap.offset + extra_off,
                       ap=[ap.ap[0]] + dims)

    with tile.TileContext(nc) as tc, ExitStack() as ctx:
        const = ctx.enter_context(tc.tile_pool(name="const", bufs=1))
        rhs_pool = ctx.enter_context(tc.tile_pool(name="rhs", bufs=4))

        SIG = mybir.ActivationFunctionType.Sigmoid

        # const loads (SP queue)
        pbf_sb = const.tile([128, 1152], bf16)
        nc.sync.dma_start(out=pbf_sb[:], in_=p_pbf[:, :])
        attnb_sb = const.tile([128, 1], f32)
        nc.sync.dma_start(out=attnb_sb[:], in_=p_attnb[:, :])
        skim_sb = const.tile([N, BPC, N], bf16)
        nc.sync.dma_start(out=skim_sb[:], in_=p_skim[:, :, :])

        w2_sb = pbf_sb[:, 0:128]
        wgt_sb = pbf_sb[:, 128:256]
        inpt_sb = pbf_sb[:, 256:1024]
        ident_sb = pbf_sb[:, 1024:1152]

        # sigmoid ACT-table warm: first ACT instruction is a sigmoid, so the
        # one table load happens during the DMA ramp, not mid-pipeline
        warm_sb = const.tile([OUT_F, 1], bf16)
        nc.scalar.activation(out=warm_sb[:], in_=attnb_sb[:], func=SIG)

        # big moving-tensor loads
        rts = [None] * BPC

        def load_rt(b, split=False):
            rt = rhs_pool.tile([128, HALF], bf16, tag="rt")
            if split:
                # chunk-granular load so the first matmuls start earlier
                for h in range(2):
                    for k in range(3):
                        c0 = k * 1536
                        nc.gpsimd.dma_start(
                            out=rt[64 * h:64 * h + OP_D + 1, c0:c0 + 1536],
                            in_=p_op4[b, h][:, c0:c0 + 1536])
            else:
                nc.gpsimd.dma_start(out=rt[0:OP_D + 1, :], in_=p_op4[b, 0])
                nc.gpsimd.dma_start(out=rt[64:64 + OP_D + 1, :], in_=p_op4[b, 1])
            rts[b] = rt

        load_rt(0, split=True)

        stbf_sb = const.tile([OUT_F, NB], bf16)       # support^T in bf16
        snat_sb = const.tile([N, BPC, OUT_F], bf16)   # support natural [j, b, d]

        pz = ctx.enter_context(tc.tile_pool(name="pz", bufs=2, space="PSUM"))
        ptr = ctx.enter_context(tc.tile_pool(name="ptr", bufs=2, space="PSUM"))

        def zslot():
            zt = pz.tile([OUT_F, 1536], f32, tag="z")
            return zt

        # ---------------- pre-phase: support, S1 ----------------
        # support^T [d, (b,j)] = weight^T @ inputs^T
        stp = zslot()
        nc.tensor.matmul(stp[:, 0:512], lhsT=wgt_sb,
                         rhs=inpt_sb[:, 0:512], start=True, stop=True)
        nc.tensor.matmul(stp[:, 512:NB], lhsT=wgt_sb,
                         rhs=inpt_sb[:, 512:NB], start=True, stop=True)
        nc.scalar.copy(out=stbf_sb[:], in_=stp[:, 0:NB])

        def emit_snat(b):
            # support natural [j, d] per b (stationary for the S1 matmul)
            pn = ptr.tile([128, 128], f32, tag="pt")
            nc.tensor.matmul(pn[:N, 0:OUT_F], lhsT=inpt_sb[:, b * N:(b + 1) * N],
                             rhs=wgt_sb, start=True, stop=True)
            nc.scalar.copy(out=snat_sb[:, b, :], in_=pn[:N, 0:OUT_F])

        load_rt(1)
        load_rt(2)
        load_rt(3)

        # ---------------- main loop over batches ----------------
        sig_pool = ctx.enter_context(tc.tile_pool(name="sig", bufs=3))
        prod_pool = ctx.enter_context(tc.tile_pool(name="prod", bufs=3))
        l1_pool = ctx.enter_context(tc.tile_pool(name="l1", bufs=2))
        l2_pool = ctx.enter_context(tc.tile_pool(name="l2", bufs=2))
        l3_pool = ctx.enter_context(tc.tile_pool(name="l3", bufs=2))
        l4_pool = ctx.enter_context(tc.tile_pool(name="l4", bufs=2))
        ofin = ctx.enter_context(tc.tile_pool(name="ofin", bufs=1))

        out_fin = ofin.tile([OUT_F, NB], bf16)

        NH = N // 2  # 48 i-rows per half

        for b in range(BPC):
            if b + 4 < BPC:
                load_rt(b + 4)
            rt = rts[b]

            st_b = stbf_sb[:, b * N:(b + 1) * N]
            l1 = l1_pool.tile([OUT_F, N * 48], bf16)

            for h in range(2):
                pbase = 64 * h
                sig_t = sig_pool.tile([OUT_F, HALF], bf16)
                for k in range(3):
                    pzt = zslot()
                    for s in range(3):
                        cc = k * 1536 + s * 512
                        nc.tensor.matmul(
                            pzt[:, s * 512:(s + 1) * 512],
                            lhsT=w2_sb[pbase:pbase + OP_D + 1, :],
                            rhs=rt[pbase:pbase + OP_D + 1, cc:cc + 512],
                            start=True, stop=True)
                    nc.scalar.activation(out=sig_t[:, k * 1536:(k + 1) * 1536],
                                         in_=pzt[:], func=SIG,
                                         bias=attnb_sb[:], scale=1.0)

                # prod[d, (i, j)] = sigma * support_bcast; Pool takes the
                # trailing POOL_MULT_COLS of the h=1 half to relieve DVE
                prod = prod_pool.tile([OUT_F, HALF], bf16)
                pmc = POOL_MULT_COLS if h == 1 else 0
                dvc = HALF - pmc
                ndv = dvc // N
                st_bcast = bass.AP(tensor=st_b.tensor, offset=st_b.offset,
                                   ap=[st_b.ap[0], [0, ndv], st_b.ap[1]])
                nc.vector.tensor_tensor(out=prod[:, 0:dvc],
                                        in0=sig_t[:, 0:dvc],
                                        in1=st_bcast, op=MUL)
                if pmc:
                    st_bcast_p = bass.AP(tensor=st_b.tensor, offset=st_b.offset,
                                         ap=[st_b.ap[0], [0, pmc // N],
                                             st_b.ap[1]])
                    nc.gpsimd.tensor_tensor(out=prod[:, dvc:HALF],
                                            in0=sig_t[:, dvc:HALF],
                                            in1=st_bcast_p, op=MUL)

                # L1: j 96 -> 48 (per half, into the shared batch tile);
                # the trailing POOL_L1_GROUPS i-groups go to Pool
                ng = NH - POOL_L1_GROUPS
                nc.vector.tensor_tensor(
                    out=l1[:, h * NH * 48:h * NH * 48 + ng * 48],
                    in0=sub_ap(prod[:], 0, [[N, ng], [1, 48]]),
                    in1=sub_ap(prod[:], 48, [[N, ng], [1, 48]]),
                    op=ADD)
                if POOL_L1_GROUPS:
                    nc.gpsimd.tensor_tensor(
                        out=l1[:, h * NH * 48 + ng * 48:(h + 1) * NH * 48],
                        in0=sub_ap(prod[:], ng * N, [[N, POOL_L1_GROUPS], [1, 48]]),
                        in1=sub_ap(prod[:], ng * N + 48,
                                   [[N, POOL_L1_GROUPS], [1, 48]]),
                        op=ADD)

                if b == 0 and h == 0:
                    # pre-phase tail: snat matmuls + copies fill early ACT
                    # bubbles without gating the first sigmoids
                    for bb in range(BPC):
                        emit_snat(bb)

            # remaining DVE levels once per batch over all 96 i-groups
            l2 = l2_pool.tile([OUT_F, N * 24], bf16)
            nc.vector.tensor_tensor(
                out=l2[:],
                in0=sub_ap(l1[:], 0, [[48, N], [1, 24]]),
                in1=sub_ap(l1[:], 24, [[48, N], [1, 24]]),
                op=ADD)
            if TAIL_FROM == "l2":
                tail_t, tail_w = l2, 24
            else:
                l3 = l3_pool.tile([OUT_F, N * 12], bf16)
                nc.vector.tensor_tensor(
                    out=l3[:],
                    in0=sub_ap(l2[:], 0, [[24, N], [1, 12]]),
                    in1=sub_ap(l2[:], 12, [[24, N], [1, 12]]),
                    op=ADD)
                if TAIL_FROM == "l3":
                    tail_t, tail_w = l3, 12
                else:
                    l4 = l4_pool.tile([OUT_F, N * 6], bf16)
                    nc.vector.tensor_tensor(
                        out=l4[:],
                        in0=sub_ap(l3[:], 0, [[12, N], [1, 6]]),
                        in1=sub_ap(l3[:], 6, [[12, N], [1, 6]]),
                        op=ADD)
                    tail_t, tail_w = l4, 6

            # PE tail: PSUM-accumulate S1 (skip+identity term) plus the
            # remaining tail_w strided views of the tree level
            o0 = b * N
            ps = ptr.tile([128, 128], f32, tag="pt")
            nc.tensor.matmul(ps[:, 0:N], lhsT=snat_sb[:, b, :],
                             rhs=skim_sb[:, b, :], start=True, stop=False)
            for v in range(tail_w):
                nc.tensor.matmul(ps[:, 0:N], lhsT=ident_sb,
                                 rhs=sub_ap(tail_t[:], v, [[tail_w, N]]),
                                 start=False, stop=(v == tail_w - 1),
                                 skip_group_check=True)
            nc.scalar.copy(out=out_fin[:, o0:o0 + N], in_=ps[:, 0:N])

            # store this batch's output columns
            nc.sync.dma_start(out=p_out[:, b * N:(b + 1) * N],
                              in_=out_fin[:, b * N:(b + 1) * N])

    nc.finalize()
    return nc


def _get_nc():
    if "nc" not in _CACHE:
        _CACHE["nc"] = _build_nc()
    return _CACHE["nc"]


def marshal_core(inputs, adj, op_emb, weight, attn_w, attn_b, self_op_emb, core):
    """Build the in_map for one core (layout/dtype marshaling + mask logits)."""
    from ml_dtypes import bfloat16

    sl = slice(core * BPC, (core + 1) * BPC)
    op_sh = np.array(op_emb[sl], np.float32)              # [BPC, N, N, OP_D]
    idx = np.arange(N)
    op_sh[:, idx, idx, :] = np.asarray(self_op_emb, np.float32)
    op_t = op_sh.transpose(0, 3, 1, 2)                    # [BPC, OP_D, N(i), N(j)]
    adj_sh = np.asarray(adj[sl]).astype(np.int32)         # [BPC, N, N]
    eye = np.eye(N, dtype=np.float32)
    # mask logit row: -100 where (adj + I) in {0, 1} else 0
    adjp = adj_sh.astype(np.float32) + eye
    m2 = np.where(adjp <= 1.0, np.float32(NEG), np.float32(0.0))  # [BPC, N, N]
    op4 = np.empty((BPC, 2, OP_D + 1, HALF), bfloat16)
    op4[:, :, :OP_D, :] = op_t.reshape(BPC, OP_D, 2, HALF).transpose(
        0, 2, 1, 3).astype(bfloat16)
    op4[:, :, OP_D, :] = m2.reshape(BPC, 2, HALF).astype(bfloat16)
    # skim[j, b, i] = (adjP == 1) + I, i.e. skip edges + residual identity
    skim = ((adjp == 1.0).astype(np.float32) + eye)        # [BPC, N(i), N(j)]
    skim = np.ascontiguousarray(skim.transpose(2, 0, 1)).astype(bfloat16)
    inpt = np.ascontiguousarray(
        np.asarray(inputs[sl], np.float32).reshape(BPC * N, IN_F).T)

    w2 = np.zeros((128, 128), np.float32)
    w2[0:OP_D] = attn_w
    w2[OP_D] = 1.0
    w2[64:64 + OP_D] = attn_w
    w2[64 + OP_D] = 1.0

    pbf = np.zeros((128, 1152), bfloat16)
    pbf[:, 0:128] = w2.astype(bfloat16)
    pbf[:, 128:256] = np.asarray(weight, np.float32).astype(bfloat16)
    pbf[:, 256:1024] = inpt.astype(bfloat16)
    pbf[:, 1024:1152] = np.eye(128, dtype=np.float32).astype(bfloat16)
    attnb = np.asarray(attn_b, np.float32).reshape(128, 1)

    return {
        "op4": op4,
        "skim": skim,
        "pbf": pbf,
        "attnb": attnb,
    }


def _ensure_ntff_hook():
    """Provide antenv.axon_hooks if the image lacks it (NTFF timing under axon)."""
    import sys as _sys

    try:
        from antenv.axon_hooks import get_axon_ntff_profile_hook  # noqa: F401
        return
    except ImportError:
        pass

    import contextlib
    import ctypes
    import types

    so_path = "/opt/axon/libaxon_pjrt.so"
    try:
        lib = ctypes.CDLL(so_path)
    except OSError:
        lib = None
    if lib is None or not hasattr(lib, "axon_start_nrt_profile"):
        hook = None
    else:
        lib.axon_start_nrt_profile.argtypes = [
            ctypes.POINTER(ctypes.c_int64), ctypes.c_size_t]
        lib.axon_start_nrt_profile.restype = ctypes.c_int64
        lib.axon_stop_nrt_profile.argtypes = [ctypes.c_char_p]
        lib.axon_stop_nrt_profile.restype = ctypes.c_int64

        @contextlib.contextmanager
        def hook(output_dir, device_ids):
            import jax
            jax.devices()
            if device_ids:
                ids = (ctypes.c_int64 * len(device_ids))(*device_ids)
                rc = lib.axon_start_nrt_profile(ids, len(device_ids))
            else:
                rc = lib.axon_start_nrt_profile(None, 0)
            if rc != 0:
                raise RuntimeError(f"axon_start_nrt_profile rc={rc}")
            try:
                yield
            finally:
                n = lib.axon_stop_nrt_profile(str(output_dir).encode())
                print(f"ntff profile: {n} file(s) written to {output_dir}")

    mod = types.ModuleType("antenv.axon_hooks")
    _state = {"hook": hook}
    mod.get_axon_ntff_profile_hook = lambda: _state["hook"]

    def _set(h):
        _state["hook"] = h

    mod.set_axon_ntff_profile_hook = _set
    _sys.modules["antenv.axon_hooks"] = mod


def run(inputs, adj, op_emb, weight, attn_w, attn_b, self_op_emb, trace=False):
    if trace:
        _ensure_ntff_hook()
    from concourse.bass_utils import run_bass_kernel_spmd

    nc = _get_nc()
    in_maps = [
        marshal_core(inputs, adj, op_emb, weight, attn_w, attn_b, self_op_emb, c)
        for c in range(NCORES)
    ]
    res = run_bass_kernel_spmd(nc, in_maps, core_ids=list(range(NCORES)), trace=trace)
    # out is [F, BPC*N] bf16 per core; undo the transpose on host
    out = np.concatenate(
        [np.asarray(res.results[c]["out"], np.float32)
         .reshape(OUT_F, BPC, N).transpose(1, 2, 0)
         for c in range(NCORES)], axis=0)
    return np.ascontiguousarray(out, np.float32), res


def kernel(inputs, adj, op_emb, weight, attn_w, attn_b, self_op_emb):
    out, _ = run(inputs, adj, op_emb, weight, attn_w, attn_b, self_op_emb, trace=False)
    return out


# revision 26
# speedup vs baseline: 1.1790x; 1.0676x over previous
"""Trainium2 Bass kernel for DenseGraphSimpleOpEdgeFlow (GNN message passing).

Reference semantics (per batch b):
  support = inputs @ weight                                    [N, F]
  op_emb[diag] = self_op_emb; adjP = adj + I
  attn = sigmoid(op_emb @ attn_w + attn_b)                     [N, N, F]
  attn = (adjP != 0) * attn;  attn = 1 where adjP == 1 (skip)
  out[i, :] = sum_j attn[i, j, :] * support[j, :] + support[i, :]

Sharding: data-parallel over batch B=64 across 8 cores (8 batches/core).

Per-core device mapping (engine-balanced around the ACT sigmoid wall):
  * z = W_aug^T @ op_emb^T on TensorE, bf16 moving tensor marshaled on the
    host (49 rows = 48 op dims + one mask-logit row in {0,-100} which drives
    sigmoid to 0 for masked and skip edges). Two 49-row halves at partition
    bases 0/64 cover each batch's 9216 edge columns.
  * sigmoid on ScalarE straight out of PSUM (bias = attn_b per partition),
    bf16 out. 48 x 1536-col instructions: this engine is the wall.
  * DVE (2x bf16): prod = sigma * support_bcast per half (4608 cols), then
    tree levels L1 (96j->48) and L2 (48->24).
  * Pool (GpSimd): tree tail L3/L4/L5 (24->12->6->3) and the three final
    48-col adds (3-way cred + s1), writing bf16 out_fin.
  * skip/identity term S1[d,(b,i)] = support^T @ (skipmask+I) as small
    TensorE matmuls; skim mask is host-marshaled (bf16), so no on-device
    mask arithmetic at all.
  * output stays [d, (b,i)] bf16 and is DMA'd per batch; the host undoes
    the transpose. No on-device output transposes.
  * all DMA triggers issue from the Sync (SP) queue, keeping Pool free.
"""

import numpy as np

B, N, IN_F, OUT_F, OP_D = 64, 96, 128, 128, 48
NCORES = 8
BPC = B // NCORES  # batches per core
HALF = (N // 2) * N  # 4608 columns per half
NEG = -100.0

_CACHE = {}

# columns of each batch's multiply handled by Pool (0 disables)
POOL_MULT_COLS = 0
# i-groups (of 48 per half) of the L1 tree level handled by Pool.
# NOTE: any Pool compute poisons DVE 2x throughput (shared SBUF port pair),
# so this must stay 0 while DVE is the bottleneck.
POOL_L1_GROUPS = 0
# tree level whose remaining j-width is accumulated on the PE via
# identity-matmuls into PSUM ('l2' = 24 views, 'l3' = 12, 'l4' = 6)
TAIL_FROM = "l4"


def _build_nc():
    import concourse.bass as bass
    import concourse.bacc as bacc
    import concourse.tile as tile
    from concourse import mybir
    from contextlib import ExitStack

    f32 = mybir.dt.float32
    bf16 = mybir.dt.bfloat16
    MUL = mybir.AluOpType.mult
    ADD = mybir.AluOpType.add

    nc = bacc.Bacc(None, target_bir_lowering=False)

    # ---- DRAM parameters (per-core shard views, host-marshaled) ----
    # op4[b, h] is [49, HALF] bf16: rows 0-47 = op_emb^T (diagonal replaced
    # host-side), row 48 = mask logit row in {0, -100}.
    p_op4 = nc.declare_dram_parameter("op4", [BPC, 2, OP_D + 1, HALF], bf16,
                                      isOutput=False)
    # skim[j, b, i] = (adjP == 1) + I : skip edges plus residual identity
    p_skim = nc.declare_dram_parameter("skim", [N, BPC, N], bf16, isOutput=False)
    # packed constants: pbf = [w2 | wgt | inpt | ident]
    p_pbf = nc.declare_dram_parameter("pbf", [128, 1152], bf16, isOutput=False)
    p_attnb = nc.declare_dram_parameter("attnb", [128, 1], f32, isOutput=False)
    p_out = nc.declare_dram_parameter("out", [OUT_F, BPC * N], bf16, isOutput=True)

    NB = BPC * N  # 768

    def sub_ap(ap, extra_off, dims):
        return bass.AP(tensor=ap.tensor, offset=ap.offset + extra_off,
                       ap=[ap.ap[0]] + dims)

    with tile.TileContext(nc) as tc, ExitStack() as ctx:
        const = ctx.enter_context(tc.tile_pool(name="const", bufs=1))
        rhs_pool = ctx.enter_context(tc.tile_pool(name="rhs", bufs=4))

        SIG = mybir.ActivationFunctionType.Sigmoid

        # const loads (SP queue); attnb first so the ACT table warm starts
        # as early as possible
        attnb_sb = const.tile([128, 1], f32)
        nc.sync.dma_start(out=attnb_sb[:], in_=p_attnb[:, :])
        pbf_sb = const.tile([128, 1152], bf16)
        nc.sync.dma_start(out=pbf_sb[:], in_=p_pbf[:, :])
        skim_sb = const.tile([N, BPC, N], bf16)
        nc.sync.dma_start(out=skim_sb[:], in_=p_skim[:, :, :])

        w2_sb = pbf_sb[:, 0:128]
        wgt_sb = pbf_sb[:, 128:256]
        inpt_sb = pbf_sb[:, 256:1024]
        ident_sb = pbf_sb[:, 1024:1152]

        # sigmoid ACT-table warm: first ACT instruction is a sigmoid, so the
        # one table load happens during the DMA ramp, not mid-pipeline
        warm_sb = const.tile([OUT_F, 1], bf16)
        nc.scalar.activation(out=warm_sb[:], in_=attnb_sb[:], func=SIG)

        # big moving-tensor loads
        rts = [None] * BPC

        def load_rt(b, split=False):
            rt = rhs_pool.tile([128, HALF], bf16, tag="rt")
            if split:
                # chunk-granular load so the first matmuls start earlier
                for h in range(2):
                    for k in range(3):
                        c0 = k * 1536
                        nc.gpsimd.dma_start(
                            out=rt[64 * h:64 * h + OP_D + 1, c0:c0 + 1536],
                            in_=p_op4[b, h][:, c0:c0 + 1536])
            else:
                nc.gpsimd.dma_start(out=rt[0:OP_D + 1, :], in_=p_op4[b, 0])
                nc.gpsimd.dma_start(out=rt[64:64 + OP_D + 1, :], in_=p_op4[b, 1])
            rts[b] = rt

        load_rt(0, split=True)

        stbf_sb = const.tile([OUT_F, NB], bf16)       # support^T in bf16
        snat_sb = const.tile([N, BPC, OUT_F], bf16)   # support natural [j, b, d]

        pz = ctx.enter_context(tc.tile_pool(name="pz", bufs=2, space="PSUM"))
        ptr = ctx.enter_context(tc.tile_pool(name="ptr", bufs=2, space="PSUM"))

        def zslot():
            zt = pz.tile([OUT_F, 1536], f32, tag="z")
            return zt

        # ---------------- pre-phase: support, S1 ----------------
        # support^T [d, (b,j)] = weight^T @ inputs^T
        stp = zslot()
        nc.tensor.matmul(stp[:, 0:512], lhsT=wgt_sb,
                         rhs=inpt_sb[:, 0:512], start=True, stop=True)
        nc.tensor.matmul(stp[:, 512:NB], lhsT=wgt_sb,
                         rhs=inpt_sb[:, 512:NB], start=True, stop=True)
        nc.scalar.copy(out=stbf_sb[:], in_=stp[:, 0:NB])

        def emit_snat(b):
            # support natural [j, d] per b (stationary for the S1 matmul)
            pn = ptr.tile([128, 128], f32, tag="pt")
            nc.tensor.matmul(pn[:N, 0:OUT_F], lhsT=inpt_sb[:, b * N:(b + 1) * N],
                             rhs=wgt_sb, start=True, stop=True)
            nc.scalar.copy(out=snat_sb[:, b, :], in_=pn[:N, 0:OUT_F])

        load_rt(1)
        load_rt(2)
        load_rt(3)

        # ---------------- main loop over batches ----------------
        sig_pool = ctx.enter_context(tc.tile_pool(name="sig", bufs=3))
        prod_pool = ctx.enter_context(tc.tile_pool(name="prod", bufs=3))
        l1_pool = ctx.enter_context(tc.tile_pool(name="l1", bufs=2))
        l2_pool = ctx.enter_context(tc.tile_pool(name="l2", bufs=2))
        l3_pool = ctx.enter_context(tc.tile_pool(name="l3", bufs=2))
        l4_pool = ctx.enter_context(tc.tile_pool(name="l4", bufs=2))
        ofin = ctx.enter_context(tc.tile_pool(name="ofin", bufs=1))

        out_fin = ofin.tile([OUT_F, NB], bf16)

        NH = N // 2  # 48 i-rows per half

        for b in range(BPC):
            if b + 4 < BPC:
                load_rt(b + 4)
            rt = rts[b]

            st_b = stbf_sb[:, b * N:(b + 1) * N]
            l1 = l1_pool.tile([OUT_F, N * 48], bf16)

            for h in range(2):
                pbase = 64 * h
                sig_t = sig_pool.tile([OUT_F, HALF], bf16)
                for k in range(3):
                    pzt = zslot()
                    for s in range(3):
                        cc = k * 1536 + s * 512
                        nc.tensor.matmul(
                            pzt[:, s * 512:(s + 1) * 512],
                            lhsT=w2_sb[pbase:pbase + OP_D + 1, :],
                            rhs=rt[pbase:pbase + OP_D + 1, cc:cc + 512],
                            start=True, stop=True)
                    nc.scalar.activation(out=sig_t[:, k * 1536:(k + 1) * 1536],
                                         in_=pzt[:], func=SIG,
                                         bias=attnb_sb[:], scale=1.0)

                # prod[d, (i, j)] = sigma * support_bcast; Pool takes the
                # trailing POOL_MULT_COLS of the h=1 half to relieve DVE
                prod = prod_pool.tile([OUT_F, HALF], bf16)
                pmc = POOL_MULT_COLS if h == 1 else 0
                dvc = HALF - pmc
                ndv = dvc // N
                st_bcast = bass.AP(tensor=st_b.tensor, offset=st_b.offset,
                                   ap=[st_b.ap[0], [0, ndv], st_b.ap[1]])
                nc.vector.tensor_tensor(out=prod[:, 0:dvc],
                                        in0=sig_t[:, 0:dvc],
                                        in1=st_bcast, op=MUL)
                if pmc:
                    st_bcast_p = bass.AP(tensor=st_b.tensor, offset=st_b.offset,
                                         ap=[st_b.ap[0], [0, pmc // N],
                                             st_b.ap[1]])
                    nc.gpsimd.tensor_tensor(out=prod[:, dvc:HALF],
                                            in0=sig_t[:, dvc:HALF],
                                            in1=st_bcast_p, op=MUL)

                # L1: j 96 -> 48 (per half, into the shared batch tile);
                # the trailing POOL_L1_GROUPS i-groups go to Pool
                ng = NH - POOL_L1_GROUPS
                nc.vector.tensor_tensor(
                    out=l1[:, h * NH * 48:h * NH * 48 + ng * 48],
                    in0=sub_ap(prod[:], 0, [[N, ng], [1, 48]]),
                    in1=sub_ap(prod[:], 48, [[N, ng], [1, 48]]),
                    op=ADD)
                if POOL_L1_GROUPS:
                    nc.gpsimd.tensor_tensor(
                        out=l1[:, h * NH * 48 + ng * 48:(h + 1) * NH * 48],
                        in0=sub_ap(prod[:], ng * N, [[N, POOL_L1_GROUPS], [1, 48]]),
                        in1=sub_ap(prod[:], ng * N + 48,
                                   [[N, POOL_L1_GROUPS], [1, 48]]),
                        op=ADD)

                if b == 0 and h == 0:
                    # pre-phase tail: snat matmuls + copies fill early ACT
                    # bubbles without gating the first sigmoids
                    for bb in range(BPC):
                        emit_snat(bb)

            # remaining DVE levels once per batch over all 96 i-groups
            l2 = l2_pool.tile([OUT_F, N * 24], bf16)
            nc.vector.tensor_tensor(
                out=l2[:],
                in0=sub_ap(l1[:], 0, [[48, N], [1, 24]]),
                in1=sub_ap(l1[:], 24, [[48, N], [1, 24]]),
                op=ADD)
            if TAIL_FROM == "l2":
                tail_t, tail_w = l2, 24
            else:
                l3 = l3_pool.tile([OUT_F, N * 12], bf16)
                nc.vector.tensor_tensor(
                    out=l3[:],
                    in0=sub_ap(l2[:], 0, [[24, N], [1, 12]]),
                    in1=sub_ap(l2[:], 12, [[24, N], [1, 12]]),
                    op=ADD)
                if TAIL_FROM == "l3":
                    tail_t, tail_w = l3, 12
                else:
                    l4 = l4_pool.tile([OUT_F, N * 6], bf16)
                    nc.vector.tensor_tensor(
                        out=l4[:],
                        in0=sub_ap(l3[:], 0, [[12, N], [1, 6]]),
                        in1=sub_ap(l3[:], 6, [[12, N], [1, 6]]),
                        op=ADD)
                    tail_t, tail_w = l4, 6

            # PE tail: PSUM-accumulate S1 (skip+identity term) plus the
            # remaining tail_w strided views of the tree level
            o0 = b * N
            ps = ptr.tile([128, 128], f32, tag="pt")
            nc.tensor.matmul(ps[:, 0:N], lhsT=snat_sb[:, b, :],
                             rhs=skim_sb[:, b, :], start=True, stop=False)
            for v in range(tail_w):
                nc.tensor.matmul(ps[:, 0:N], lhsT=ident_sb,
                                 rhs=sub_ap(tail_t[:], v, [[tail_w, N]]),
                                 start=False, stop=(v == tail_w - 1),
                                 skip_group_check=True)
            nc.scalar.copy(out=out_fin[:, o0:o0 + N], in_=ps[:, 0:N])

            # store this batch's output columns
            nc.sync.dma_start(out=p_out[:, b * N:(b + 1) * N],
                              in_=out_fin[:, b * N:(b + 1) * N])

    nc.finalize()
    return nc


def _get_nc():
    if "nc" not in _CACHE:
        _CACHE["nc"] = _build_nc()
    return _CACHE["nc"]


def marshal_core(inputs, adj, op_emb, weight, attn_w, attn_b, self_op_emb, core):
    """Build the in_map for one core (layout/dtype marshaling + mask logits)."""
    from ml_dtypes import bfloat16

    sl = slice(core * BPC, (core + 1) * BPC)
    op_sh = np.array(op_emb[sl], np.float32)              # [BPC, N, N, OP_D]
    idx = np.arange(N)
    op_sh[:, idx, idx, :] = np.asarray(self_op_emb, np.float32)
    op_t = op_sh.transpose(0, 3, 1, 2)                    # [BPC, OP_D, N(i), N(j)]
    adj_sh = np.asarray(adj[sl]).astype(np.int32)         # [BPC, N, N]
    eye = np.eye(N, dtype=np.float32)
    # mask logit row: -100 where (adj + I) in {0, 1} else 0
    adjp = adj_sh.astype(np.float32) + eye
    m2 = np.where(adjp <= 1.0, np.float32(NEG), np.float32(0.0))  # [BPC, N, N]
    op4 = np.empty((BPC, 2, OP_D + 1, HALF), bfloat16)
    op4[:, :, :OP_D, :] = op_t.reshape(BPC, OP_D, 2, HALF).transpose(
        0, 2, 1, 3).astype(bfloat16)
    op4[:, :, OP_D, :] = m2.reshape(BPC, 2, HALF).astype(bfloat16)
    # skim[j, b, i] = (adjP == 1) + I, i.e. skip edges + residual identity
    skim = ((adjp == 1.0).astype(np.float32) + eye)        # [BPC, N(i), N(j)]
    skim = np.ascontiguousarray(skim.transpose(2, 0, 1)).astype(bfloat16)
    inpt = np.ascontiguousarray(
        np.asarray(inputs[sl], np.float32).reshape(BPC * N, IN_F).T)

    w2 = np.zeros((128, 128), np.float32)
    w2[0:OP_D] = attn_w
    w2[OP_D] = 1.0
    w2[64:64 + OP_D] = attn_w
    w2[64 + OP_D] = 1.0

    pbf = np.zeros((128, 1152), bfloat16)
    pbf[:, 0:128] = w2.astype(bfloat16)
    pbf[:, 128:256] = np.asarray(weight, np.float32).astype(bfloat16)
    pbf[:, 256:1024] = inpt.astype(bfloat16)
    pbf[:, 1024:1152] = np.eye(128, dtype=np.float32).astype(bfloat16)
    attnb = np.asarray(attn_b, np.float32).reshape(128, 1)

    return {
        "op4": op4,
        "skim": skim,
        "pbf": pbf,
        "attnb": attnb,
    }


def _ensure_ntff_hook():
    """Provide antenv.axon_hooks if the image lacks it (NTFF timing under axon)."""
    import sys as _sys

    try:
        from antenv.axon_hooks import get_axon_ntff_profile_hook  # noqa: F401
        return
    except ImportError:
        pass

    import contextlib
    import ctypes
    import types

    so_path = "/opt/axon/libaxon_pjrt.so"
    try:
        lib = ctypes.CDLL(so_path)
    except OSError:
        lib = None
    if lib is None or not hasattr(lib, "axon_start_nrt_profile"):
        hook = None
    else:
        lib.axon_start_nrt_profile.argtypes = [
            ctypes.POINTER(ctypes.c_int64), ctypes.c_size_t]
        lib.axon_start_nrt_profile.restype = ctypes.c_int64
        lib.axon_stop_nrt_profile.argtypes = [ctypes.c_char_p]
        lib.axon_stop_nrt_profile.restype = ctypes.c_int64

        @contextlib.contextmanager
        def hook(output_dir, device_ids):
            import jax
            jax.devices()
            if device_ids:
                ids = (ctypes.c_int64 * len(device_ids))(*device_ids)
                rc = lib.axon_start_nrt_profile(ids, len(device_ids))
            else:
                rc = lib.axon_start_nrt_profile(None, 0)
            if rc != 0:
                raise RuntimeError(f"axon_start_nrt_profile rc={rc}")
            try:
                yield
            finally:
                n = lib.axon_stop_nrt_profile(str(output_dir).encode())
                print(f"ntff profile: {n} file(s) written to {output_dir}")

    mod = types.ModuleType("antenv.axon_hooks")
    _state = {"hook": hook}
    mod.get_axon_ntff_profile_hook = lambda: _state["hook"]

    def _set(h):
        _state["hook"] = h

    mod.set_axon_ntff_profile_hook = _set
    _sys.modules["antenv.axon_hooks"] = mod


def run(inputs, adj, op_emb, weight, attn_w, attn_b, self_op_emb, trace=False):
    if trace:
        _ensure_ntff_hook()
    from concourse.bass_utils import run_bass_kernel_spmd

    nc = _get_nc()
    in_maps = [
        marshal_core(inputs, adj, op_emb, weight, attn_w, attn_b, self_op_emb, c)
        for c in range(NCORES)
    ]
    res = run_bass_kernel_spmd(nc, in_maps, core_ids=list(range(NCORES)), trace=trace)
    # out is [F, BPC*N] bf16 per core; undo the transpose on host
    out = np.concatenate(
        [np.asarray(res.results[c]["out"], np.float32)
         .reshape(OUT_F, BPC, N).transpose(1, 2, 0)
         for c in range(NCORES)], axis=0)
    return np.ascontiguousarray(out, np.float32), res


def kernel(inputs, adj, op_emb, weight, attn_w, attn_b, self_op_emb):
    out, _ = run(inputs, adj, op_emb, weight, attn_w, attn_b, self_op_emb, trace=False)
    return out
